# revision 6
# baseline (speedup 1.0000x reference)
"""Trainium2 Bass kernel for the SNN Net (antenna-fuse -> hidden -> LIF scan
-> time-fuse -> output -> softmax), data-parallel over 8 NeuronCores.

Self-contained: hardcodes shapes/sharding; builds the Bass/Tile program and
runs it via run_bass_kernel_spmd.

v2 design notes (per core, bs=256 rows = 2 chunks of 128 partitions):
- x is streamed in 10 large DMAs of [128, 18*4096B] (72 KB contiguous per
  partition row) instead of 360 DMAs of 4 KB rows: ~30x fewer descriptors,
  each 18x larger.
- antenna fuse: qA = x[aA]*rA + x[pA], qB = x[aB]*rB + x[pB] on DVE,
  fused = qB*cc + qA on GpSimd; hidden matmul via PE transpose (fp32) of
  fused into PSUM, ACT copy to SBUF, then 2 accumulating matmuls against
  wpp [128d, 10h] plus a K=1 ones-row bias matmul.
- LIF scan on DVE in [128b, 20(c,h)] layout, reading sn straight from PSUM;
  spikes are scaled by w_time[t] via tensor_scalar immediates (no wt_rep
  const tensor needed).
- head: time-reduce + output linear + softmax on DVE/ACT; result is PE-
  transposed to [4,128] so the output DMA is 4 contiguous descriptors.
"""

import os
import sys
from contextlib import ExitStack

import numpy as np

for _p in ("/opt/trn_rl_repo", "/root/.axon_site/_ro/trn_rl_repo"):
    if _p not in sys.path and os.path.isdir(_p):
        sys.path.insert(0, _p)

import concourse.bacc as bacc
import concourse.bass as bass
import concourse.mybir as mybir
import concourse.tile as tile
from concourse.bass_utils import run_bass_kernel_spmd

F32 = mybir.dt.float32
ALU = mybir.AluOpType

B, T, A, D, H, O = 2048, 90, 4, 256, 10, 2
N_CORES = 8
BS = B // N_CORES          # 256 batch rows per core
NB = BS // 128             # 2 row-chunks of 128 partitions
CHW = NB * H               # 20 free elems per scan step
BETA = 0.95
THR = 1.0
TG = 18                    # timesteps per x DMA (72 KB per partition row)
NG = T // TG               # 5 groups
SG = 2                     # timesteps per antenna-fuse DVE op / PSUM pair


def _pick_pairs(w_ant):
    """Order the 4 antennas into two (pivot, other) pairs so the global
    max-|w| antenna is the pivot of pair A. Returns indices and folded
    scalars (rA, rB, cc, base) with |rA|,|rB|,|cc| <= 1 and base = w[pA]."""
    w = np.asarray(w_ant, np.float64)
    order = np.argsort(-np.abs(w))
    pA, aA = int(order[0]), int(order[3])
    pB, aB = int(order[1]), int(order[2])
    base = float(w[pA])

    def safe_div(n, d):
        return float(n / d) if abs(d) > 0.0 else 0.0

    rA = safe_div(w[aA], w[pA])
    rB = safe_div(w[aB], w[pB])
    cc = safe_div(w[pB], w[pA])
    return (pA, aA, pB, aB), (rA, rB, cc, base)


def _build(sc, bs=BS, t_steps=T):
    """Emit the Bass program. sc: dict of host-folded scalars/lists."""
    rA, rB, cc = sc["rA"], sc["rB"], sc["cc"]
    pA, aA, pB, aB = sc["idx"]
    w_time = sc["w_time"]          # list of 90 floats (immediates)
    b_time = sc["b_time"]
    b_out = sc["b_out"]

    nb = bs // 128
    assert bs % 128 == 0 and nb == 2, "kernel assumes 256 rows/core"
    assert t_steps == NG * TG and TG % SG == 0

    nc = bacc.Bacc()
    x_d = nc.dram_tensor("x", (bs, t_steps * A * D), F32, kind="ExternalInput")
    wppT_d = nc.dram_tensor("wppT", (CHW, 128), F32, kind="ExternalInput")
    wb_d = nc.dram_tensor("wb", (1, 64), F32, kind="ExternalInput")
    out_d = nc.dram_tensor("out", (nb * O, 128), F32, kind="ExternalOutput")

    with ExitStack() as ctx:
        tc = ctx.enter_context(tile.TileContext(nc))
        consts = ctx.enter_context(tc.tile_pool(name="consts", bufs=1))
        xp = ctx.enter_context(tc.tile_pool(name="xp", bufs=2))
        qp = ctx.enter_context(tc.tile_pool(name="qp", bufs=2))
        ftp = ctx.enter_context(tc.tile_pool(name="ftp", bufs=3))
        state = ctx.enter_context(tc.tile_pool(name="state", bufs=2))
        spkp = ctx.enter_context(tc.tile_pool(name="spk", bufs=1))
        outp = ctx.enter_context(tc.tile_pool(name="outp", bufs=1))
        ps_ft = ctx.enter_context(tc.tile_pool(name="ps_ft", bufs=3, space="PSUM"))
        ps_sn = ctx.enter_context(tc.tile_pool(name="ps_sn", bufs=2, space="PSUM"))
        ps_ms = ctx.enter_context(tc.tile_pool(name="ps_ms", bufs=1, space="PSUM"))

        # ---- constants ----
        ident = consts.tile([128, 128], F32)
        from concourse.masks import make_identity
        make_identity(nc, ident)

        ones1 = consts.tile([1, 128], F32)
        nc.vector.memset(ones1, 1.0)

        wb = consts.tile([1, 64], F32)
        nc.sync.dma_start(out=wb, in_=wb_d[:, :])
        bcomb = wb[0:1, 0:H]

        wppT = consts.tile([CHW, 128], F32)
        nc.sync.dma_start(out=wppT, in_=wppT_d[:, :])
        # wpp[p, k*H+h] = w_hid[h, k*128+p] * base   (PE transpose of wppT)
        wpp_ps = ps_ms.tile([128, CHW], F32, tag="misc")
        nc.tensor.matmul(wpp_ps, lhsT=wppT, rhs=ident[0:CHW, 0:CHW],
                         is_transpose=True, start=True, stop=True,
                         skip_group_check=True)
        wpp = consts.tile([128, CHW], F32)
        nc.scalar.copy(out=wpp, in_=wpp_ps)

        # broadcast w_out row to all partitions: [128, O*CHW]
        woutb_ps = ps_ms.tile([128, O * CHW], F32, tag="misc")
        nc.tensor.matmul(woutb_ps, lhsT=ones1, rhs=wb[0:1, H:H + O * CHW],
                         start=True, stop=True, skip_group_check=True)
        woutb = consts.tile([128, O * CHW], F32)
        nc.scalar.copy(out=woutb, in_=woutb_ps)

        spk = spkp.tile([128, t_steps * CHW], F32)

        mem = state.tile([128, CHW], F32, tag="mem")
        nc.vector.memset(mem, 0.0)

        sn_tiles = {}
        for g in range(NG):
            for c in range(nb):
                x_flat = xp.tile([128, TG * A * D], F32, tag="x")
                src = x_d[c * 128:(c + 1) * 128,
                          g * TG * A * D:(g + 1) * TG * A * D]
                # alternate the two HWDGE rings (SP / ACT) for the x stream
                dma_eng = nc.sync if (g * nb + c) % 2 == 0 else nc.scalar
                dma_eng.dma_start(out=x_flat, in_=src)
                x_t = x_flat[:].rearrange("p (t a d) -> p t a d", t=TG, a=A, d=D)

                sn = ps_sn.tile([128, TG * H], F32, tag=f"sn{c}")
                sn_tiles[c] = sn
                for sb in range(TG // SG):
                    t0 = sb * SG
                    qA = qp.tile([128, SG, D], F32, tag="qA")
                    nc.vector.scalar_tensor_tensor(
                        out=qA, in0=x_t[:, t0:t0 + SG, aA], scalar=rA,
                        in1=x_t[:, t0:t0 + SG, pA], op0=ALU.mult, op1=ALU.add)
                    qB = qp.tile([128, SG, D], F32, tag="qB")
                    nc.vector.scalar_tensor_tensor(
                        out=qB, in0=x_t[:, t0:t0 + SG, aB], scalar=rB,
                        in1=x_t[:, t0:t0 + SG, pB], op0=ALU.mult, op1=ALU.add)
                    qBc = qp.tile([128, SG, D], F32, tag="qBc")
                    nc.scalar.mul(qBc, qB, cc)
                    fused = qp.tile([128, SG, D], F32, tag="fused")
                    nc.gpsimd.tensor_tensor(out=fused, in0=qBc, in1=qA,
                                            op=ALU.add)
                    # transpose the SG timesteps (2 halves each) into PSUM
                    ftps = ps_ft.tile([128, SG * D], F32, tag="ftps")
                    for tl in range(SG):
                        for h2 in range(2):
                            nc.tensor.matmul(
                                ftps[:, (tl * 2 + h2) * 128:(tl * 2 + h2 + 1) * 128],
                                lhsT=fused[:, tl, h2 * 128:(h2 + 1) * 128],
                                rhs=ident, is_transpose=True,
                                start=True, stop=True, skip_group_check=True)
                    fT = ftp.tile([128, SG * D], F32, tag="fT")
                    nc.scalar.copy(out=fT, in_=ftps)
                    for tl in range(SG):
                        sl = sn[:, (t0 + tl) * H:(t0 + tl + 1) * H]
                        nc.tensor.matmul(sl, lhsT=ones1, rhs=bcomb,
                                         start=True, stop=False,
                                         skip_group_check=True)
                        for h2 in range(2):
                            nc.tensor.matmul(
                                sl,
                                lhsT=fT[:, (tl * 2 + h2) * 128:(tl * 2 + h2 + 1) * 128],
                                rhs=wpp[:, h2 * H:(h2 + 1) * H],
                                start=False, stop=(h2 == 1),
                                skip_group_check=True)
            # ---- LIF scan over this group's timesteps ----
            for tl in range(TG):
                t = g * TG + tl
                u = state.tile([128, CHW], F32, tag="u")
                for c in range(nb):
                    nc.vector.scalar_tensor_tensor(
                        out=u[:, c * H:(c + 1) * H], in0=mem[:, c * H:(c + 1) * H],
                        scalar=BETA, in1=sn_tiles[c][:, tl * H:(tl + 1) * H],
                        op0=ALU.mult, op1=ALU.add)
                mem_new = state.tile([128, CHW], F32, tag="mem")
                nc.vector.scalar_tensor_tensor(
                    out=mem_new, in0=mem, scalar=THR, in1=u,
                    op0=ALU.is_le, op1=ALU.mult)
                nc.vector.tensor_scalar(
                    out=spk[:, t * CHW:(t + 1) * CHW], in0=mem_new,
                    scalar1=THR, scalar2=w_time[t],
                    op0=ALU.is_gt, op1=ALU.mult)
                mem = mem_new

        # ---- time-fuse + output head + softmax ----
        ft = outp.tile([128, CHW], F32)
        spk_v = spk[:].rearrange("p (t f) -> p f t", f=CHW)
        nc.vector.tensor_reduce(out=ft, in_=spk_v, axis=mybir.AxisListType.X,
                                op=ALU.add)
        nc.vector.tensor_scalar_add(out=ft, in0=ft, scalar1=b_time)
        lg = outp.tile([128, O * nb], F32)          # cols o*nb + c
        for o in range(O):
            mo = outp.tile([128, CHW], F32, tag="mo")
            nc.vector.tensor_tensor(out=mo, in0=ft,
                                    in1=woutb[:, o * CHW:(o + 1) * CHW],
                                    op=ALU.mult)
            nc.vector.tensor_reduce(
                out=lg[:, o * nb:(o + 1) * nb],
                in_=mo[:].rearrange("p (c h) -> p c h", h=H),
                axis=mybir.AxisListType.X, op=ALU.add)
            nc.vector.tensor_scalar_add(
                out=lg[:, o * nb:(o + 1) * nb],
                in0=lg[:, o * nb:(o + 1) * nb], scalar1=b_out[o])
        ex = outp.tile([128, O * nb], F32)
        nc.scalar.activation(out=ex, in_=lg,
                             func=mybir.ActivationFunctionType.Exp)
        ssum = outp.tile([128, nb], F32)
        nc.vector.tensor_tensor(out=ssum, in0=ex[:, 0:nb],
                                in1=ex[:, nb:2 * nb], op=ALU.add)
        rec = outp.tile([128, nb], F32)
        nc.vector.reciprocal(out=rec, in_=ssum)
        res = outp.tile([128, nb * O], F32)         # cols c*O + o
        for c in range(nb):
            for o in range(O):
                nc.vector.tensor_tensor(
                    out=res[:, c * O + o: c * O + o + 1],
                    in0=ex[:, o * nb + c: o * nb + c + 1],
                    in1=rec[:, c: c + 1], op=ALU.mult)
        # transpose to [4, 128] so the output DMA is 4 contiguous rows
        resT_ps = ps_ms.tile([nb * O, 128], F32, tag="misc")
        nc.tensor.matmul(resT_ps, lhsT=res, rhs=ident, is_transpose=True,
                         start=True, stop=True, skip_group_check=True)
        resT = outp.tile([nb * O, 128], F32)
        nc.scalar.copy(out=resT, in_=resT_ps)
        nc.sync.dma_start(out=out_d[:, :], in_=resT)
    nc.finalize()
    return nc


def _prep_weights(w_ant, b_ant, w_hid, b_hid, w_time, b_time, w_out, b_out):
    """Host-side weight folding. Returns (scalars, const_arrays)."""
    w_ant = np.asarray(w_ant, np.float32)
    w_hid = np.asarray(w_hid, np.float32)
    w_out = np.asarray(w_out, np.float32)
    idx, (rA, rB, cc, base) = _pick_pairs(w_ant)
    # wppT[k*H+h, p] = w_hid[h, k*128+p] * base
    wppT = np.empty((CHW, 128), np.float32)
    for k in range(NB):
        wppT[k * H:(k + 1) * H, :] = (w_hid[:, k * 128:(k + 1) * 128]
                                      * np.float32(base))
    b_comb = (np.float32(b_ant) * w_hid.sum(axis=1)
              + np.asarray(b_hid, np.float32)).astype(np.float32)
    wb = np.zeros((1, 64), np.float32)
    wb[0, 0:H] = b_comb
    # wout row: [o*CHW + c*H + h] = w_out[o, h]
    wb[0, H:H + O * CHW] = np.concatenate(
        [np.tile(w_out[o], NB) for o in range(O)])
    scalars = {"rA": rA, "rB": rB, "cc": cc, "idx": idx,
               "w_time": [float(v) for v in np.asarray(w_time, np.float32)],
               "b_time": float(np.float32(b_time)),
               "b_out": [float(v) for v in np.asarray(b_out, np.float32)]}
    consts = {"wppT": wppT, "wb": wb}
    return scalars, consts


_CACHE = {}


def kernel(x, w_ant, b_ant, w_hid, b_hid, w_time, b_time, w_out, b_out):
    x = np.ascontiguousarray(np.asarray(x, np.float32))
    assert x.shape == (B, T, A, D), x.shape
    scalars, consts = _prep_weights(w_ant, b_ant, w_hid, b_hid, w_time,
                                    b_time, w_out, b_out)
    key = (scalars["rA"], scalars["rB"], scalars["cc"], scalars["idx"],
           tuple(scalars["w_time"]), scalars["b_time"],
           tuple(scalars["b_out"]))
    nc = _CACHE.get(key)
    if nc is None:
        nc = _build(scalars, BS, T)
        _CACHE[key] = nc
    in_maps = []
    for i in range(N_CORES):
        m = {"x": np.ascontiguousarray(x[i * BS:(i + 1) * BS])}
        m.update(consts)
        in_maps.append(m)
    r = run_bass_kernel_spmd(nc, in_maps, core_ids=list(range(N_CORES)))
    out = np.empty((B, O), np.float32)
    for i in range(N_CORES):
        arr = r.results[i]["out"]          # [nb*O, 128], rows c*O + o
        for c in range(NB):
            blk = arr[c * O:(c + 1) * O, :]            # [O, 128]
            out[i * BS + c * 128:i * BS + (c + 1) * 128, :] = blk.T
    return out


# revision 10
# speedup vs baseline: 1.0227x; 1.0227x over previous
"""Trainium2 Bass kernel for the SNN Net (antenna-fuse -> hidden -> LIF scan
-> time-fuse -> output -> softmax), data-parallel over 8 NeuronCores.

Self-contained: hardcodes shapes/sharding; builds the Bass/Tile program and
runs it via run_bass_kernel_spmd.

v2 design notes (per core, bs=256 rows = 2 chunks of 128 partitions):
- x is streamed in 10 large DMAs of [128, 18*4096B] (72 KB contiguous per
  partition row) instead of 360 DMAs of 4 KB rows: ~30x fewer descriptors,
  each 18x larger.
- antenna fuse: qA = x[aA]*rA + x[pA], qB = x[aB]*rB + x[pB] on DVE,
  fused = qB*cc + qA on GpSimd; hidden matmul via PE transpose (fp32) of
  fused into PSUM, ACT copy to SBUF, then 2 accumulating matmuls against
  wpp [128d, 10h] plus a K=1 ones-row bias matmul.
- LIF scan on DVE in [128b, 20(c,h)] layout, reading sn straight from PSUM;
  spikes are scaled by w_time[t] via tensor_scalar immediates (no wt_rep
  const tensor needed).
- head: time-reduce + output linear + softmax on DVE/ACT; result is PE-
  transposed to [4,128] so the output DMA is 4 contiguous descriptors.
"""

import os
import sys
from contextlib import ExitStack

import numpy as np

for _p in ("/opt/trn_rl_repo", "/root/.axon_site/_ro/trn_rl_repo"):
    if _p not in sys.path and os.path.isdir(_p):
        sys.path.insert(0, _p)

import concourse.bacc as bacc
import concourse.bass as bass
import concourse.mybir as mybir
import concourse.tile as tile
from concourse.bass_utils import run_bass_kernel_spmd

F32 = mybir.dt.float32
ALU = mybir.AluOpType

B, T, A, D, H, O = 2048, 90, 4, 256, 10, 2
N_CORES = 8
BS = B // N_CORES          # 256 batch rows per core
NB = BS // 128             # 2 row-chunks of 128 partitions
CHW = NB * H               # 20 free elems per scan step
BETA = 0.95
THR = 1.0
TG = 18                    # timesteps per x DMA (72 KB per partition row)
NG = T // TG               # 5 groups
SG = 2                     # timesteps per antenna-fuse DVE op / PSUM pair


def _pick_pairs(w_ant):
    """Order the 4 antennas into two (pivot, other) pairs so the global
    max-|w| antenna is the pivot of pair A. Returns indices and folded
    scalars (rA, rB, cc, base) with |rA|,|rB|,|cc| <= 1 and base = w[pA]."""
    w = np.asarray(w_ant, np.float64)
    order = np.argsort(-np.abs(w))
    pA, aA = int(order[0]), int(order[3])
    pB, aB = int(order[1]), int(order[2])
    base = float(w[pA])

    def safe_div(n, d):
        return float(n / d) if abs(d) > 0.0 else 0.0

    rA = safe_div(w[aA], w[pA])
    rB = safe_div(w[aB], w[pB])
    cc = safe_div(w[pB], w[pA])
    return (pA, aA, pB, aB), (rA, rB, cc, base)


def _build(sc, bs=BS, t_steps=T, qp_bufs=2, ftp_bufs=3, split_first=False):
    """Emit the Bass program. sc: dict of host-folded scalars/lists."""
    rA, rB, cc = sc["rA"], sc["rB"], sc["cc"]
    pA, aA, pB, aB = sc["idx"]
    w_time = sc["w_time"]          # list of 90 floats (immediates)
    b_time = sc["b_time"]
    b_out = sc["b_out"]

    nb = bs // 128
    assert bs % 128 == 0 and nb == 2, "kernel assumes 256 rows/core"
    assert t_steps == NG * TG and TG % SG == 0

    nc = bacc.Bacc()
    x_d = nc.dram_tensor("x", (bs, t_steps * A * D), F32, kind="ExternalInput")
    wppT_d = nc.dram_tensor("wppT", (CHW, 128), F32, kind="ExternalInput")
    wb_d = nc.dram_tensor("wb", (1, 64), F32, kind="ExternalInput")
    out_d = nc.dram_tensor("out", (nb * O, 128), F32, kind="ExternalOutput")

    with ExitStack() as ctx:
        tc = ctx.enter_context(tile.TileContext(nc))
        consts = ctx.enter_context(tc.tile_pool(name="consts", bufs=1))
        xp = ctx.enter_context(tc.tile_pool(name="xp", bufs=2))
        qp = ctx.enter_context(tc.tile_pool(name="qp", bufs=qp_bufs))
        ftp = ctx.enter_context(tc.tile_pool(name="ftp", bufs=ftp_bufs))
        state = ctx.enter_context(tc.tile_pool(name="state", bufs=2))
        spkp = ctx.enter_context(tc.tile_pool(name="spk", bufs=1))
        outp = ctx.enter_context(tc.tile_pool(name="outp", bufs=1))
        ps_ft = ctx.enter_context(tc.tile_pool(name="ps_ft", bufs=3, space="PSUM"))
        ps_sn = ctx.enter_context(tc.tile_pool(name="ps_sn", bufs=2, space="PSUM"))
        ps_ms = ctx.enter_context(tc.tile_pool(name="ps_ms", bufs=1, space="PSUM"))

        # ---- constants ----
        ident = consts.tile([128, 128], F32)
        from concourse.masks import make_identity
        make_identity(nc, ident)

        ones1 = consts.tile([1, 128], F32)
        nc.vector.memset(ones1, 1.0)

        wb = consts.tile([1, 64], F32)
        nc.sync.dma_start(out=wb, in_=wb_d[:, :])
        bcomb = wb[0:1, 0:H]

        wppT = consts.tile([CHW, 128], F32)
        nc.sync.dma_start(out=wppT, in_=wppT_d[:, :])
        # wpp[p, k*H+h] = w_hid[h, k*128+p] * base   (PE transpose of wppT)
        wpp_ps = ps_ms.tile([128, CHW], F32, tag="misc")
        nc.tensor.matmul(wpp_ps, lhsT=wppT, rhs=ident[0:CHW, 0:CHW],
                         is_transpose=True, start=True, stop=True,
                         skip_group_check=True)
        wpp = consts.tile([128, CHW], F32)
        nc.scalar.copy(out=wpp, in_=wpp_ps)

        # broadcast w_out row to all partitions: [128, O*CHW]
        woutb_ps = ps_ms.tile([128, O * CHW], F32, tag="misc")
        nc.tensor.matmul(woutb_ps, lhsT=ones1, rhs=wb[0:1, H:H + O * CHW],
                         start=True, stop=True, skip_group_check=True)
        woutb = consts.tile([128, O * CHW], F32)
        nc.scalar.copy(out=woutb, in_=woutb_ps)

        spk = spkp.tile([128, t_steps * CHW], F32)

        mem = state.tile([128, CHW], F32, tag="mem")
        nc.vector.memset(mem, 0.0)

        sn_tiles = {}
        for g in range(NG):
            for c in range(nb):
                x_flat = xp.tile([128, TG * A * D], F32, tag="x")
                src = x_d[c * 128:(c + 1) * 128,
                          g * TG * A * D:(g + 1) * TG * A * D]
                # alternate the two HWDGE rings (SP / ACT) for the x stream
                dma_eng = nc.sync if (g * nb + c) % 2 == 0 else nc.scalar
                if split_first and g == 0:
                    half = TG * A * D // 2
                    dma_eng.dma_start(out=x_flat[:, 0:half],
                                      in_=src[:, 0:half])
                    dma_eng.dma_start(out=x_flat[:, half:],
                                      in_=src[:, half:])
                else:
                    dma_eng.dma_start(out=x_flat, in_=src)
                x_t = x_flat[:].rearrange("p (t a d) -> p t a d", t=TG, a=A, d=D)

                sn = ps_sn.tile([128, TG * H], F32, tag=f"sn{c}")
                sn_tiles[c] = sn
                for sb in range(TG // SG):
                    t0 = sb * SG
                    qA = qp.tile([128, SG, D], F32, tag="qA")
                    nc.vector.scalar_tensor_tensor(
                        out=qA, in0=x_t[:, t0:t0 + SG, aA], scalar=rA,
                        in1=x_t[:, t0:t0 + SG, pA], op0=ALU.mult, op1=ALU.add)
                    qB = qp.tile([128, SG, D], F32, tag="qB")
                    nc.vector.scalar_tensor_tensor(
                        out=qB, in0=x_t[:, t0:t0 + SG, aB], scalar=rB,
                        in1=x_t[:, t0:t0 + SG, pB], op0=ALU.mult, op1=ALU.add)
                    qBc = qp.tile([128, SG, D], F32, tag="qBc")
                    nc.scalar.mul(qBc, qB, cc)
                    fused = qp.tile([128, SG, D], F32, tag="fused")
                    nc.gpsimd.tensor_tensor(out=fused, in0=qBc, in1=qA,
                                            op=ALU.add)
                    # transpose the SG timesteps (2 halves each) into PSUM
                    ftps = ps_ft.tile([128, SG * D], F32, tag="ftps")
                    for tl in range(SG):
                        for h2 in range(2):
                            nc.tensor.matmul(
                                ftps[:, (tl * 2 + h2) * 128:(tl * 2 + h2 + 1) * 128],
                                lhsT=fused[:, tl, h2 * 128:(h2 + 1) * 128],
                                rhs=ident, is_transpose=True,
                                start=True, stop=True, skip_group_check=True)
                    fT = ftp.tile([128, SG * D], F32, tag="fT")
                    nc.scalar.copy(out=fT, in_=ftps)
                    for tl in range(SG):
                        sl = sn[:, (t0 + tl) * H:(t0 + tl + 1) * H]
                        nc.tensor.matmul(sl, lhsT=ones1, rhs=bcomb,
                                         start=True, stop=False,
                                         skip_group_check=True)
                        for h2 in range(2):
                            nc.tensor.matmul(
                                sl,
                                lhsT=fT[:, (tl * 2 + h2) * 128:(tl * 2 + h2 + 1) * 128],
                                rhs=wpp[:, h2 * H:(h2 + 1) * H],
                                start=False, stop=(h2 == 1),
                                skip_group_check=True)
            # ---- LIF scan over this group's timesteps ----
            for tl in range(TG):
                t = g * TG + tl
                u = state.tile([128, CHW], F32, tag="u")
                for c in range(nb):
                    nc.vector.scalar_tensor_tensor(
                        out=u[:, c * H:(c + 1) * H], in0=mem[:, c * H:(c + 1) * H],
                        scalar=BETA, in1=sn_tiles[c][:, tl * H:(tl + 1) * H],
                        op0=ALU.mult, op1=ALU.add)
                mem_new = state.tile([128, CHW], F32, tag="mem")
                nc.vector.scalar_tensor_tensor(
                    out=mem_new, in0=mem, scalar=THR, in1=u,
                    op0=ALU.is_le, op1=ALU.mult)
                nc.vector.tensor_scalar(
                    out=spk[:, t * CHW:(t + 1) * CHW], in0=mem_new,
                    scalar1=THR, scalar2=w_time[t],
                    op0=ALU.is_gt, op1=ALU.mult)
                mem = mem_new

        # ---- time-fuse + output head + softmax ----
        ft = outp.tile([128, CHW], F32)
        spk_v = spk[:].rearrange("p (t f) -> p f t", f=CHW)
        nc.vector.tensor_reduce(out=ft, in_=spk_v, axis=mybir.AxisListType.X,
                                op=ALU.add)
        nc.vector.tensor_scalar_add(out=ft, in0=ft, scalar1=b_time)
        lg = outp.tile([128, O * nb], F32)          # cols o*nb + c
        for o in range(O):
            mo = outp.tile([128, CHW], F32, tag="mo")
            nc.vector.tensor_tensor(out=mo, in0=ft,
                                    in1=woutb[:, o * CHW:(o + 1) * CHW],
                                    op=ALU.mult)
            nc.vector.tensor_reduce(
                out=lg[:, o * nb:(o + 1) * nb],
                in_=mo[:].rearrange("p (c h) -> p c h", h=H),
                axis=mybir.AxisListType.X, op=ALU.add)
            nc.vector.tensor_scalar_add(
                out=lg[:, o * nb:(o + 1) * nb],
                in0=lg[:, o * nb:(o + 1) * nb], scalar1=b_out[o])
        ex = outp.tile([128, O * nb], F32)
        nc.scalar.activation(out=ex, in_=lg,
                             func=mybir.ActivationFunctionType.Exp)
        ssum = outp.tile([128, nb], F32)
        nc.vector.tensor_tensor(out=ssum, in0=ex[:, 0:nb],
                                in1=ex[:, nb:2 * nb], op=ALU.add)
        rec = outp.tile([128, nb], F32)
        nc.vector.reciprocal(out=rec, in_=ssum)
        res = outp.tile([128, nb * O], F32)         # cols c*O + o
        for c in range(nb):
            for o in range(O):
                nc.vector.tensor_tensor(
                    out=res[:, c * O + o: c * O + o + 1],
                    in0=ex[:, o * nb + c: o * nb + c + 1],
                    in1=rec[:, c: c + 1], op=ALU.mult)
        # transpose to [4, 128] so the output DMA is 4 contiguous rows
        resT_ps = ps_ms.tile([nb * O, 128], F32, tag="misc")
        nc.tensor.matmul(resT_ps, lhsT=res, rhs=ident, is_transpose=True,
                         start=True, stop=True, skip_group_check=True)
        resT = outp.tile([nb * O, 128], F32)
        nc.scalar.copy(out=resT, in_=resT_ps)
        nc.sync.dma_start(out=out_d[:, :], in_=resT)
    nc.finalize()
    return nc


def _prep_weights(w_ant, b_ant, w_hid, b_hid, w_time, b_time, w_out, b_out):
    """Host-side weight folding. Returns (scalars, const_arrays)."""
    w_ant = np.asarray(w_ant, np.float32)
    w_hid = np.asarray(w_hid, np.float32)
    w_out = np.asarray(w_out, np.float32)
    idx, (rA, rB, cc, base) = _pick_pairs(w_ant)
    # wppT[k*H+h, p] = w_hid[h, k*128+p] * base
    wppT = np.empty((CHW, 128), np.float32)
    for k in range(NB):
        wppT[k * H:(k + 1) * H, :] = (w_hid[:, k * 128:(k + 1) * 128]
                                      * np.float32(base))
    b_comb = (np.float32(b_ant) * w_hid.sum(axis=1)
              + np.asarray(b_hid, np.float32)).astype(np.float32)
    wb = np.zeros((1, 64), np.float32)
    wb[0, 0:H] = b_comb
    # wout row: [o*CHW + c*H + h] = w_out[o, h]
    wb[0, H:H + O * CHW] = np.concatenate(
        [np.tile(w_out[o], NB) for o in range(O)])
    scalars = {"rA": rA, "rB": rB, "cc": cc, "idx": idx,
               "w_time": [float(v) for v in np.asarray(w_time, np.float32)],
               "b_time": float(np.float32(b_time)),
               "b_out": [float(v) for v in np.asarray(b_out, np.float32)]}
    consts = {"wppT": wppT, "wb": wb}
    return scalars, consts


_CACHE = {}


def kernel(x, w_ant, b_ant, w_hid, b_hid, w_time, b_time, w_out, b_out):
    x = np.ascontiguousarray(np.asarray(x, np.float32))
    assert x.shape == (B, T, A, D), x.shape
    scalars, consts = _prep_weights(w_ant, b_ant, w_hid, b_hid, w_time,
                                    b_time, w_out, b_out)
    key = (scalars["rA"], scalars["rB"], scalars["cc"], scalars["idx"],
           tuple(scalars["w_time"]), scalars["b_time"],
           tuple(scalars["b_out"]))
    nc = _CACHE.get(key)
    if nc is None:
        nc = _build(scalars, BS, T, qp_bufs=3, ftp_bufs=4, split_first=True)
        _CACHE[key] = nc
    in_maps = []
    for i in range(N_CORES):
        m = {"x": np.ascontiguousarray(x[i * BS:(i + 1) * BS])}
        m.update(consts)
        in_maps.append(m)
    r = run_bass_kernel_spmd(nc, in_maps, core_ids=list(range(N_CORES)))
    out = np.empty((B, O), np.float32)
    for i in range(N_CORES):
        arr = r.results[i]["out"]          # [nb*O, 128], rows c*O + o
        for c in range(NB):
            blk = arr[c * O:(c + 1) * O, :]            # [O, 128]
            out[i * BS + c * 128:i * BS + (c + 1) * 128, :] = blk.T
    return out
